# revision 34
# baseline (speedup 1.0000x reference)
"""Trainium2 Bass kernel for nn_KGather (sparse_attention gather+scale).

Reference computation:
    out[n, p, t, w, c] = r_weight[n, p, t] * k[n, r_idx[n, p, t], w, c]
with n=16, p2=49, topk=8, w2=64, ck=128 (all fp32; r_idx int).

Strategy (8 cores, data parallel over n, 2 batch elements per core):
  - Host side: fold the gather indices AND the routing weights into a
    block-diagonal scaled one-hot matrix per core:
        onehot[j, pt] = r_weight[n_l, p, t]  if j == n_l*49 + r_idx[n_l, p, t]
    with pt = (n_l*49 + p)*8 + t, j in [0, 98); rows are zero-padded to
    128 so the input DMA uses all 16 SDMA engines.
  - Device side (static program, data-independent):
        out_core[pt, wc] = sum_j onehot[j, pt] * k_core[j, wc]
    i.e. a dense matmul on the TensorEngine.  Everything on device runs
    in bf16 (the problem tolerates rel err 2e-2; bf16 contributes ~7e-3):
    the one-hot+k input is bf16, read once, and the output is written to
    HBM as bf16 (12.8 MB/core) then upcast to fp32 on the host.  This
    halves the dominant HBM write traffic vs fp32 and runs the matmul at
    full bf16 rate (the original fp32 matmul was ~4x slower and was the
    bottleneck).  Steady state is HBM-write-bound at ~358 GB/s/core.
  - PSUM tiles are filled by 512-wide matmuls and drained (fp32->bf16
    cast) to SBUF staging; drains alternate between ScalarE and VectorE
    (~55/45 by their measured rates) so neither engine exceeds the DMA
    store time.  Stores stream per half-chunk; the first and last chunks
    store in quarters so the store pipe starts ~2us earlier and the tail
    after the final drain is short.
  - The PE clock ramps 1.2->2.4 GHz only after ~12-19us of sustained
    activity (HAM), and the cold-PE column rate is BELOW the store line
    rate, so dummy warm-up matmuls run from the earliest possible point
    (before the TileContext entry barrier) through the input-load window.
  - A BIR post-pass removes the per-matmul Ldweights reload when
    consecutive matmuls share the same stationary operand (walrus ships
    with --enable-ldw-opt=false), taking the warm matmul cadence from
    ~350ns to ~258ns per 512 columns.

Each one-hot column has exactly one nonzero, so the matmul reproduces
r_weight * k exactly up to bf16 rounding of w, k, and the product.
"""

import numpy as np
import ml_dtypes

BF16 = ml_dtypes.bfloat16

# Problem shape (hardcoded per contest rules).
N, P2, TOPK, W2, CK = 16, 49, 8, 64, 128
NCORES = 8
NB = N // NCORES          # batch elements per core = 2
KROWS = NB * P2           # data rows (contraction dim) per core = 98
ROWS = 128                # padded to 128 partitions: a 98-partition DMA
                          # leaves 2 of every 16 SDMA engines idle and
                          # measured only ~235 GB/s on the load
PT = NB * P2 * TOPK       # output windows per core = 784
WC = W2 * CK              # window elements = 8192
PT_CHUNK = 112            # 7 pt chunks of 112 (<=128 partitions)
MM_CHUNK = 512            # matmul moving free size (this walrus build
                          # rejects 1024-wide Matmult: "ISA check failed")
DR_CHUNK = 1024           # drain free size (one [112,1024] PSUM tile, 2
                          # banks).  2048-wide drains with 2 PSUM slots
                          # measured ~8us worse even with a warm PE: the
                          # coarse PSUM recycling stalls the PE.
ST_CHUNK = 4096           # store free size (two drains per half-store)
LD_SPLIT = 2048           # k columns in the first load

_PROGRAM_CACHE = {}


def _drain_engine_schedule():
    """Assign each of the 56 drains to ScalarE (ACT) or VectorE (DVE).

    Cost per [112,1024] drain: ACT (1024+172)/1.2GHz = 997ns,
    DVE (1024+120)/0.96GHz = 1192ns.  Balanced split: ACT share
    f = 1192/(997+1192) = 0.545 -> 30 ACT / 26 DVE, interleaved.
    """
    n = (PT // PT_CHUNK) * (WC // DR_CHUNK)
    # ACT (1024+172)/1.2GHz = 997ns, DVE (1024+120)/0.96GHz = 1192ns.
    n_act = round(n * 1192.0 / (997.0 + 1192.0))
    sched = []
    acc = 0
    for j in range(n):
        nxt = (j + 1) * n_act // n
        sched.append("act" if nxt > acc else "dve")
        acc = nxt
    return sched


def _build_program(patch=True):
    """Build the (data-independent) per-core Bass program.

    patch=True applies _split_multi_waits (required for the HW compile;
    the JSON round-trip breaks CoreSim, so use patch=False for sim)."""
    import concourse.bass as bass
    import concourse.mybir as mybir
    import concourse.tile as tile

    nc = bass.Bass()
    # PE warm-up part 1, BEFORE the TileContext: the PE ramps from 1.2 to
    # 2.4 GHz only after ~12-19us of sustained activity (HAM), so every
    # us of earlier PE activity moves the ramp earlier.  Raw (non-Tile)
    # dummy matmuls execute right after the PE's NEFF prologue, ~2.5us
    # before the TileContext entry barrier opens.  Inputs are
    # uninitialized garbage; the PSUM target is freed again before the
    # tile pools allocate (PE program order makes the overlap safe).
    with nc.sbuf_tensor([128, 624], mybir.dt.bfloat16) as pre_sb, \
         nc.psum_tensor([112, 512], mybir.dt.float32) as pre_ps:
        for _ in range(5):
            nc.tensor.matmul(pre_ps[:], lhsT=pre_sb[:, :112],
                             rhs=pre_sb[:, 112:624], start=True, stop=True)
    # onehot and k_core are packed into one input ([128, 784+8192]) so a
    # load covers whole partition lines.
    koh_d = nc.dram_tensor("koh", [ROWS, PT + WC], mybir.dt.bfloat16,
                           kind="ExternalInput")
    out_d = nc.dram_tensor("out_core", [PT, WC], mybir.dt.bfloat16,
                           kind="ExternalOutput")

    bf16 = mybir.dt.bfloat16
    f32 = mybir.dt.float32
    n_cp = PT // PT_CHUNK
    n_st = WC // ST_CHUNK
    dr_per_st = ST_CHUNK // DR_CHUNK
    mm_per_dr = DR_CHUNK // MM_CHUNK
    sched = _drain_engine_schedule()

    with tile.TileContext(nc) as tc:
        with (
            tc.tile_pool(name="const", bufs=1) as cpool,
            tc.tile_pool(name="stage", bufs=5) as spool,
            tc.tile_pool(name="psum", bufs=4, space="PSUM") as ppool,
        ):
            koh_sb = cpool.tile([ROWS, PT + WC], bf16)
            # PE warm-up part 2: keep the PE busy through the input-load
            # window so the HAM activity clock keeps running.  Tile's
            # dependency tracker rejects reads of never-written tiles, so
            # the inputs are memset first.
            wu_lhsT = cpool.tile([ROWS, PT_CHUNK], bf16)
            wu_rhs = cpool.tile([ROWS, MM_CHUNK], bf16)
            wu_out = cpool.tile([1, 2], f32)
            wu_ps = ppool.tile([PT_CHUNK, DR_CHUNK], f32, space="PSUM",
                               tag="ps")
            # DVE memsets (no ACT table-load dependency, unlike memzero)
            # so the dummy matmuls start as soon as the PE is up; the ACT
            # copy pre-triggers the 1.28us ACT_TABLE_LOAD off the critical
            # path of the first real drain.
            nc.vector.memset(wu_lhsT[:], 0.0)
            nc.vector.memset(wu_rhs[:], 0.0)
            nc.scalar.copy(out=wu_out[:1, 1:2], in_=wu_rhs[:1, :1])
            for _ in range(7):
                nc.tensor.matmul(wu_ps[:, :MM_CHUNK], lhsT=wu_lhsT[:],
                                 rhs=wu_rhs[:], start=True, stop=True)
            # one tiny read so the pool slot is freed for the main loop
            nc.vector.tensor_copy(out=wu_out[:1, :1], in_=wu_ps[:1, :1])
            # Split loads so matmuls can start after the first ~1 MB.
            # Both issue back-to-back on the same HWDGE ring (qSPDynamicHW)
            # and execute in FIFO order, so load-a streams at full rate
            # and load-b follows with no completion-handoff gap.  (An
            # explicit WAW serialization measured ~3us of idle DMA per
            # handoff; a 3-way serialized split was worse still.)
            # ...and the loads go on the ACT HWDGE ring (stores issue
            # from Sync): per-ring FIFO would otherwise hold the first
            # stores' packets behind load-b's tail even when their data
            # is ready ~1.5us before load-b completes.
            cuts = [0, PT + LD_SPLIT, PT + WC]
            for lo, hi in zip(cuts, cuts[1:]):
                nc.scalar.dma_start(out=koh_sb[:, lo:hi],
                                    in_=koh_d[:, lo:hi])

            di = 0
            ecost = {"act": 0.0, "dve": 0.0}
            for cp in range(n_cp):
                stage = spool.tile([PT_CHUNK, WC], bf16)
                lhsT = koh_sb[:, cp * PT_CHUNK:(cp + 1) * PT_CHUNK]
                # Finer stores on the first chunk start the store stream
                # earlier (fewer drains gate the first store); on the last
                # chunk they shorten the tail.
                st_chunk = ST_CHUNK if cp < n_cp - 1 else ST_CHUNK // 2
                for st in range(WC // st_chunk):
                    # Drains alternate engines per-PSUM-tile (tying all of
                    # one store's drains to a single engine measured ~6us
                    # worse: with 4 PSUM slots it serializes the two drain
                    # engines instead of overlapping them).
                    for dr in range(st_chunk // DR_CHUNK):
                        ps = ppool.tile([PT_CHUNK, DR_CHUNK], f32,
                                        space="PSUM", tag="ps")
                        base = st * st_chunk + dr * DR_CHUNK
                        for m in range(mm_per_dr):
                            rhs = koh_sb[:, PT + base + m * MM_CHUNK:
                                         PT + base + (m + 1) * MM_CHUNK]
                            nc.tensor.matmul(
                                ps[:, m * MM_CHUNK:(m + 1) * MM_CHUNK],
                                lhsT=lhsT, rhs=rhs, start=True, stop=True)
                        sl = slice(base, base + DR_CHUNK)
                        if sched[di] == "act":
                            nc.scalar.copy(out=stage[:, sl], in_=ps[:])
                        else:
                            nc.vector.tensor_copy(out=stage[:, sl], in_=ps[:])
                        di += 1
                    rows = slice(cp * PT_CHUNK, (cp + 1) * PT_CHUNK)
                    cols = slice(st * st_chunk, (st + 1) * st_chunk)
                    nc.sync.dma_start(out=out_d[rows, cols],
                                      in_=stage[:, cols])
    if patch:
        _dedup_ldweights(nc)
        _split_multi_waits(nc)
    return nc


def _dedup_ldweights(nc):
    """Drop the stationary operand from consecutive PE Matmults that use
    identical weights.  Walrus lowers every 2-input Matmult into an
    LDWEIGHTS+MATMUL pair (and bass passes --enable-ldw-opt=false), which
    costs ~140ns of PE time per matmul; the 16 matmuls of one pt-chunk all
    share the same lhsT, so 15 of the 16 weight loads are redundant.  A
    1-input Matmult reuses whatever the PE array already holds (the
    documented bf16 InstLdweights + non-self-loading InstMatmult pairing)."""
    import json
    from concourse import mybir

    j = json.loads(mybir.module_to_json_string(nc.m))
    ndrop = [0]
    for f in j["functions"]:
        for b in f["blocks"]:
            out = []
            prev_w = None
            for ins in b["instructions"]:
                if ins.get("engine") == "PE":
                    op = ins.get("opcode")
                    if op == "Ldweights":
                        w = json.dumps(ins["ins"], sort_keys=True)
                        if w == prev_w:
                            # redundant reload: drop it, but keep its
                            # sync as a standalone semaphore op
                            sync = ins.get("sync_info") or {}
                            if sync.get("on_wait") or sync.get("on_update"):
                                out.append({
                                    "debug": ins.get("debug", 0),
                                    "engine": "PE",
                                    "ins": [],
                                    "name": f"ldwdrop-{ins['name']}",
                                    "opcode": "EventSemaphore",
                                    "outs": [],
                                    "sync_info": sync,
                                })
                            ndrop[0] += 1
                            continue
                        prev_w = w
                    elif op not in ("Matmult", "EventSemaphore", "Nop"):
                        prev_w = None
                out.append(ins)
            b["instructions"] = out
    nc.m = mybir.parse(j)
    return ndrop[0]


def _split_multi_waits(nc):
    """This walrus build rejects >1 fused sync-wait per instruction
    ("Too many sync wait commands"). Tile's wait assigner happily fuses
    several. Rewrite the BIR: for any instruction with N>1 waits, emit
    N-1 standalone single-wait EventSemaphore instructions (same engine,
    immediately before it) and keep only the last wait fused."""
    import json
    from concourse import mybir

    j = json.loads(mybir.module_to_json_string(nc.m))
    uid = [0]
    for f in j["functions"]:
        for b in f["blocks"]:
            out = []
            for ins in b["instructions"]:
                sync = ins.get("sync_info") or {}
                waits = sync.get("on_wait") or []
                if len(waits) > 1:
                    for w in waits[:-1]:
                        uid[0] += 1
                        out.append({
                            "debug": ins.get("debug", 0),
                            "engine": ins["engine"],
                            "ins": [],
                            "name": f"wsplit-{uid[0]}-{ins['name']}",
                            "opcode": "EventSemaphore",
                            "outs": [],
                            "sync_info": {"on_update": [], "on_wait": [w]},
                        })
                    sync["on_wait"] = [waits[-1]]
                out.append(ins)
            b["instructions"] = out
    nc.m = mybir.parse(j)


def get_program():
    if "nc" not in _PROGRAM_CACHE:
        _PROGRAM_CACHE["nc"] = _build_program()
    return _PROGRAM_CACHE["nc"]


def build_in_maps(r_idx, r_weight, k):
    """Host-side sharding + preprocessing: per-core inputs for the program."""
    r_idx = np.asarray(r_idx).astype(np.int64)
    r_weight = np.asarray(r_weight).astype(BF16)
    k = np.asarray(k).astype(BF16)

    pt = np.arange(PT)
    n_l = pt // (P2 * TOPK)
    p = (pt // TOPK) % P2
    t = pt % TOPK

    in_maps = []
    for c in range(NCORES):
        n0 = c * NB
        idx = r_idx[n0:n0 + NB]
        wgt = r_weight[n0:n0 + NB]
        koh = np.zeros((ROWS, PT + WC), BF16)
        rows = n_l * P2 + idx[n_l, p, t]
        koh[rows, pt] = wgt[n_l, p, t]
        koh[:KROWS, PT:] = k[n0:n0 + NB].reshape(KROWS, WC)
        in_maps.append({"koh": koh})
    return in_maps


def run_program(in_maps, trace=False, **kwargs):
    from concourse.bass_utils import run_bass_kernel_spmd
    return run_bass_kernel_spmd(get_program(), in_maps,
                                list(range(NCORES)), trace=trace, **kwargs)


def assemble_output(results):
    out = np.empty((N, P2, TOPK, W2, CK), np.float32)
    for c in range(NCORES):
        out[c * NB:(c + 1) * NB] = np.asarray(
            results[c]["out_core"]).astype(np.float32).reshape(
            NB, P2, TOPK, W2, CK)
    return out


def kernel(r_idx, r_weight, k):
    in_maps = build_in_maps(r_idx, r_weight, k)
    res = run_program(in_maps)
    return assemble_output(res.results)


# revision 35
# speedup vs baseline: 1.0015x; 1.0015x over previous
"""Trainium2 Bass kernel for nn_KGather (sparse_attention gather+scale).

Reference computation:
    out[n, p, t, w, c] = r_weight[n, p, t] * k[n, r_idx[n, p, t], w, c]
with n=16, p2=49, topk=8, w2=64, ck=128 (all fp32; r_idx int).

Strategy (8 cores, data parallel over n, 2 batch elements per core):
  - Host side: fold the gather indices AND the routing weights into a
    block-diagonal scaled one-hot matrix per core:
        onehot[j, pt] = r_weight[n_l, p, t]  if j == n_l*49 + r_idx[n_l, p, t]
    with pt = (n_l*49 + p)*8 + t, j in [0, 98); rows are zero-padded to
    128 so the input DMA uses all 16 SDMA engines.
  - Device side (static program, data-independent):
        out_core[pt, wc] = sum_j onehot[j, pt] * k_core[j, wc]
    i.e. a dense matmul on the TensorEngine.  Everything on device runs
    in bf16 (the problem tolerates rel err 2e-2; bf16 contributes ~7e-3):
    the one-hot+k input is bf16, read once, and the output is written to
    HBM as bf16 (12.8 MB/core) then upcast to fp32 on the host.  This
    halves the dominant HBM write traffic vs fp32 and runs the matmul at
    full bf16 rate (the original fp32 matmul was ~4x slower and was the
    bottleneck).  Steady state is HBM-write-bound at ~358 GB/s/core.
  - PSUM tiles are filled by 512-wide matmuls and drained (fp32->bf16
    cast) to SBUF staging; drains alternate between ScalarE and VectorE
    (~55/45 by their measured rates) so neither engine exceeds the DMA
    store time.  Stores stream per half-chunk; the first and last chunks
    store in quarters so the store pipe starts ~2us earlier and the tail
    after the final drain is short.
  - The PE clock ramps 1.2->2.4 GHz only after ~12-19us of sustained
    activity (HAM), and the cold-PE column rate is BELOW the store line
    rate, so dummy warm-up matmuls run from the earliest possible point
    (before the TileContext entry barrier) through the input-load window.
  - A BIR post-pass removes the per-matmul Ldweights reload when
    consecutive matmuls share the same stationary operand (walrus ships
    with --enable-ldw-opt=false), taking the warm matmul cadence from
    ~350ns to ~258ns per 512 columns.

Each one-hot column has exactly one nonzero, so the matmul reproduces
r_weight * k exactly up to bf16 rounding of w, k, and the product.
"""

import numpy as np
import ml_dtypes

BF16 = ml_dtypes.bfloat16

# Problem shape (hardcoded per contest rules).
N, P2, TOPK, W2, CK = 16, 49, 8, 64, 128
NCORES = 8
NB = N // NCORES          # batch elements per core = 2
KROWS = NB * P2           # data rows (contraction dim) per core = 98
ROWS = 128                # padded to 128 partitions: a 98-partition DMA
                          # leaves 2 of every 16 SDMA engines idle and
                          # measured only ~235 GB/s on the load
PT = NB * P2 * TOPK       # output windows per core = 784
WC = W2 * CK              # window elements = 8192
PT_CHUNK = 112            # 7 pt chunks of 112 (<=128 partitions)
MM_CHUNK = 512            # matmul moving free size (this walrus build
                          # rejects 1024-wide Matmult: "ISA check failed")
DR_CHUNK = 1024           # drain free size (one [112,1024] PSUM tile, 2
                          # banks).  2048-wide drains with 2 PSUM slots
                          # measured ~8us worse even with a warm PE: the
                          # coarse PSUM recycling stalls the PE.
ST_CHUNK = 4096           # store free size (two drains per half-store)
LD_SPLIT = 4096           # k columns in the first load

_PROGRAM_CACHE = {}


def _drain_engine_schedule():
    """Assign each of the 56 drains to ScalarE (ACT) or VectorE (DVE).

    Cost per [112,1024] drain: ACT (1024+172)/1.2GHz = 997ns,
    DVE (1024+120)/0.96GHz = 1192ns.  Balanced split: ACT share
    f = 1192/(997+1192) = 0.545 -> 30 ACT / 26 DVE, interleaved.
    """
    n = (PT // PT_CHUNK) * (WC // DR_CHUNK)
    # ACT (1024+172)/1.2GHz = 997ns, DVE (1024+120)/0.96GHz = 1192ns.
    n_act = round(n * 1192.0 / (997.0 + 1192.0))
    sched = []
    acc = 0
    for j in range(n):
        nxt = (j + 1) * n_act // n
        sched.append("act" if nxt > acc else "dve")
        acc = nxt
    return sched


def _build_program(patch=True):
    """Build the (data-independent) per-core Bass program.

    patch=True applies _split_multi_waits (required for the HW compile;
    the JSON round-trip breaks CoreSim, so use patch=False for sim)."""
    import concourse.bass as bass
    import concourse.mybir as mybir
    import concourse.tile as tile

    nc = bass.Bass()
    # PE warm-up part 1, BEFORE the TileContext: the PE ramps from 1.2 to
    # 2.4 GHz only after ~12-19us of sustained activity (HAM), so every
    # us of earlier PE activity moves the ramp earlier.  Raw (non-Tile)
    # dummy matmuls execute right after the PE's NEFF prologue, ~2.5us
    # before the TileContext entry barrier opens.  Inputs are
    # uninitialized garbage; the PSUM target is freed again before the
    # tile pools allocate (PE program order makes the overlap safe).
    with nc.sbuf_tensor([128, 624], mybir.dt.bfloat16) as pre_sb, \
         nc.psum_tensor([112, 512], mybir.dt.float32) as pre_ps:
        for _ in range(5):
            nc.tensor.matmul(pre_ps[:], lhsT=pre_sb[:, :112],
                             rhs=pre_sb[:, 112:624], start=True, stop=True)
    # onehot and k_core are packed into one input ([128, 784+8192]) so a
    # load covers whole partition lines.
    koh_d = nc.dram_tensor("koh", [ROWS, PT + WC], mybir.dt.bfloat16,
                           kind="ExternalInput")
    out_d = nc.dram_tensor("out_core", [PT, WC], mybir.dt.bfloat16,
                           kind="ExternalOutput")

    bf16 = mybir.dt.bfloat16
    f32 = mybir.dt.float32
    n_cp = PT // PT_CHUNK
    n_st = WC // ST_CHUNK
    dr_per_st = ST_CHUNK // DR_CHUNK
    mm_per_dr = DR_CHUNK // MM_CHUNK
    sched = _drain_engine_schedule()

    with tile.TileContext(nc) as tc:
        with (
            tc.tile_pool(name="const", bufs=1) as cpool,
            tc.tile_pool(name="stage", bufs=5) as spool,
            tc.tile_pool(name="psum", bufs=4, space="PSUM") as ppool,
        ):
            koh_sb = cpool.tile([ROWS, PT + WC], bf16)
            # PE warm-up part 2: keep the PE busy through the input-load
            # window so the HAM activity clock keeps running.  Tile's
            # dependency tracker rejects reads of never-written tiles, so
            # the inputs are memset first.
            wu_lhsT = cpool.tile([ROWS, PT_CHUNK], bf16)
            wu_rhs = cpool.tile([ROWS, MM_CHUNK], bf16)
            wu_out = cpool.tile([1, 2], f32)
            wu_ps = ppool.tile([PT_CHUNK, DR_CHUNK], f32, space="PSUM",
                               tag="ps")
            # DVE memsets (no ACT table-load dependency, unlike memzero)
            # so the dummy matmuls start as soon as the PE is up; the ACT
            # copy pre-triggers the 1.28us ACT_TABLE_LOAD off the critical
            # path of the first real drain.
            nc.vector.memset(wu_lhsT[:], 0.0)
            nc.vector.memset(wu_rhs[:], 0.0)
            nc.scalar.copy(out=wu_out[:1, 1:2], in_=wu_rhs[:1, :1])
            for _ in range(7):
                nc.tensor.matmul(wu_ps[:, :MM_CHUNK], lhsT=wu_lhsT[:],
                                 rhs=wu_rhs[:], start=True, stop=True)
            # one tiny read so the pool slot is freed for the main loop
            nc.vector.tensor_copy(out=wu_out[:1, :1], in_=wu_ps[:1, :1])
            # Split loads so matmuls can start after the first ~1 MB.
            # Both issue back-to-back on the same HWDGE ring (qSPDynamicHW)
            # and execute in FIFO order, so load-a streams at full rate
            # and load-b follows with no completion-handoff gap.  (An
            # explicit WAW serialization measured ~3us of idle DMA per
            # handoff; a 3-way serialized split was worse still.)
            # ...and the loads go on the ACT HWDGE ring (stores issue
            # from Sync): per-ring FIFO would otherwise hold the first
            # stores' packets behind load-b's tail even when their data
            # is ready ~1.5us before load-b completes.
            cuts = [0, PT + LD_SPLIT, PT + WC]
            for lo, hi in zip(cuts, cuts[1:]):
                nc.scalar.dma_start(out=koh_sb[:, lo:hi],
                                    in_=koh_d[:, lo:hi])

            di = 0
            ecost = {"act": 0.0, "dve": 0.0}
            for cp in range(n_cp):
                stage = spool.tile([PT_CHUNK, WC], bf16)
                lhsT = koh_sb[:, cp * PT_CHUNK:(cp + 1) * PT_CHUNK]
                # Finer stores on the first chunk start the store stream
                # earlier (fewer drains gate the first store); on the last
                # chunk they shorten the tail.
                st_chunk = ST_CHUNK if cp < n_cp - 1 else ST_CHUNK // 2
                for st in range(WC // st_chunk):
                    # Drains alternate engines per-PSUM-tile (tying all of
                    # one store's drains to a single engine measured ~6us
                    # worse: with 4 PSUM slots it serializes the two drain
                    # engines instead of overlapping them).
                    for dr in range(st_chunk // DR_CHUNK):
                        ps = ppool.tile([PT_CHUNK, DR_CHUNK], f32,
                                        space="PSUM", tag="ps")
                        base = st * st_chunk + dr * DR_CHUNK
                        for m in range(mm_per_dr):
                            rhs = koh_sb[:, PT + base + m * MM_CHUNK:
                                         PT + base + (m + 1) * MM_CHUNK]
                            nc.tensor.matmul(
                                ps[:, m * MM_CHUNK:(m + 1) * MM_CHUNK],
                                lhsT=lhsT, rhs=rhs, start=True, stop=True)
                        sl = slice(base, base + DR_CHUNK)
                        if sched[di] == "act":
                            nc.scalar.copy(out=stage[:, sl], in_=ps[:])
                        else:
                            nc.vector.tensor_copy(out=stage[:, sl], in_=ps[:])
                        di += 1
                    rows = slice(cp * PT_CHUNK, (cp + 1) * PT_CHUNK)
                    cols = slice(st * st_chunk, (st + 1) * st_chunk)
                    nc.sync.dma_start(out=out_d[rows, cols],
                                      in_=stage[:, cols])
    if patch:
        _dedup_ldweights(nc)
        _split_multi_waits(nc)
    return nc


def _dedup_ldweights(nc):
    """Drop the stationary operand from consecutive PE Matmults that use
    identical weights.  Walrus lowers every 2-input Matmult into an
    LDWEIGHTS+MATMUL pair (and bass passes --enable-ldw-opt=false), which
    costs ~140ns of PE time per matmul; the 16 matmuls of one pt-chunk all
    share the same lhsT, so 15 of the 16 weight loads are redundant.  A
    1-input Matmult reuses whatever the PE array already holds (the
    documented bf16 InstLdweights + non-self-loading InstMatmult pairing)."""
    import json
    from concourse import mybir

    j = json.loads(mybir.module_to_json_string(nc.m))
    ndrop = [0]
    for f in j["functions"]:
        for b in f["blocks"]:
            out = []
            prev_w = None
            for ins in b["instructions"]:
                if ins.get("engine") == "PE":
                    op = ins.get("opcode")
                    if op == "Ldweights":
                        w = json.dumps(ins["ins"], sort_keys=True)
                        if w == prev_w:
                            # redundant reload: drop it, but keep its
                            # sync as a standalone semaphore op
                            sync = ins.get("sync_info") or {}
                            if sync.get("on_wait") or sync.get("on_update"):
                                out.append({
                                    "debug": ins.get("debug", 0),
                                    "engine": "PE",
                                    "ins": [],
                                    "name": f"ldwdrop-{ins['name']}",
                                    "opcode": "EventSemaphore",
                                    "outs": [],
                                    "sync_info": sync,
                                })
                            ndrop[0] += 1
                            continue
                        prev_w = w
                    elif op not in ("Matmult", "EventSemaphore", "Nop"):
                        prev_w = None
                out.append(ins)
            b["instructions"] = out
    nc.m = mybir.parse(j)
    return ndrop[0]


def _split_multi_waits(nc):
    """This walrus build rejects >1 fused sync-wait per instruction
    ("Too many sync wait commands"). Tile's wait assigner happily fuses
    several. Rewrite the BIR: for any instruction with N>1 waits, emit
    N-1 standalone single-wait EventSemaphore instructions (same engine,
    immediately before it) and keep only the last wait fused."""
    import json
    from concourse import mybir

    j = json.loads(mybir.module_to_json_string(nc.m))
    uid = [0]
    for f in j["functions"]:
        for b in f["blocks"]:
            out = []
            for ins in b["instructions"]:
                sync = ins.get("sync_info") or {}
                waits = sync.get("on_wait") or []
                if len(waits) > 1:
                    for w in waits[:-1]:
                        uid[0] += 1
                        out.append({
                            "debug": ins.get("debug", 0),
                            "engine": ins["engine"],
                            "ins": [],
                            "name": f"wsplit-{uid[0]}-{ins['name']}",
                            "opcode": "EventSemaphore",
                            "outs": [],
                            "sync_info": {"on_update": [], "on_wait": [w]},
                        })
                    sync["on_wait"] = [waits[-1]]
                out.append(ins)
            b["instructions"] = out
    nc.m = mybir.parse(j)


def get_program():
    if "nc" not in _PROGRAM_CACHE:
        _PROGRAM_CACHE["nc"] = _build_program()
    return _PROGRAM_CACHE["nc"]


def build_in_maps(r_idx, r_weight, k):
    """Host-side sharding + preprocessing: per-core inputs for the program."""
    r_idx = np.asarray(r_idx).astype(np.int64)
    r_weight = np.asarray(r_weight).astype(BF16)
    k = np.asarray(k).astype(BF16)

    pt = np.arange(PT)
    n_l = pt // (P2 * TOPK)
    p = (pt // TOPK) % P2
    t = pt % TOPK

    in_maps = []
    for c in range(NCORES):
        n0 = c * NB
        idx = r_idx[n0:n0 + NB]
        wgt = r_weight[n0:n0 + NB]
        koh = np.zeros((ROWS, PT + WC), BF16)
        rows = n_l * P2 + idx[n_l, p, t]
        koh[rows, pt] = wgt[n_l, p, t]
        koh[:KROWS, PT:] = k[n0:n0 + NB].reshape(KROWS, WC)
        in_maps.append({"koh": koh})
    return in_maps


def run_program(in_maps, trace=False, **kwargs):
    from concourse.bass_utils import run_bass_kernel_spmd
    return run_bass_kernel_spmd(get_program(), in_maps,
                                list(range(NCORES)), trace=trace, **kwargs)


def assemble_output(results):
    out = np.empty((N, P2, TOPK, W2, CK), np.float32)
    for c in range(NCORES):
        out[c * NB:(c + 1) * NB] = np.asarray(
            results[c]["out_core"]).astype(np.float32).reshape(
            NB, P2, TOPK, W2, CK)
    return out


def kernel(r_idx, r_weight, k):
    in_maps = build_in_maps(r_idx, r_weight, k)
    res = run_program(in_maps)
    return assemble_output(res.results)


# revision 36
# speedup vs baseline: 1.0365x; 1.0349x over previous
"""Trainium2 Bass kernel for nn_KGather (sparse_attention gather+scale).

Reference computation:
    out[n, p, t, w, c] = r_weight[n, p, t] * k[n, r_idx[n, p, t], w, c]
with n=16, p2=49, topk=8, w2=64, ck=128 (all fp32; r_idx int).

Strategy (8 cores, data parallel over n, 2 batch elements per core):
  - Host side: fold the gather indices AND the routing weights into a
    block-diagonal scaled one-hot matrix per core:
        onehot[j, pt] = r_weight[n_l, p, t]  if j == n_l*49 + r_idx[n_l, p, t]
    with pt = (n_l*49 + p)*8 + t, j in [0, 98); rows are zero-padded to
    128 so the input DMA uses all 16 SDMA engines.
  - Device side (static program, data-independent):
        out_core[pt, wc] = sum_j onehot[j, pt] * k_core[j, wc]
    i.e. a dense matmul on the TensorEngine.  Everything on device runs
    in bf16 (the problem tolerates rel err 2e-2; bf16 contributes ~7e-3):
    the one-hot+k input is bf16, read once, and the output is written to
    HBM as bf16 (12.8 MB/core) then upcast to fp32 on the host.  This
    halves the dominant HBM write traffic vs fp32 and runs the matmul at
    full bf16 rate (the original fp32 matmul was ~4x slower and was the
    bottleneck).  Steady state is HBM-write-bound at ~358 GB/s/core.
  - PSUM tiles are filled by 512-wide matmuls and drained (fp32->bf16
    cast) to SBUF staging; drains alternate between ScalarE and VectorE
    (~55/45 by their measured rates) so neither engine exceeds the DMA
    store time.  Stores stream per half-chunk; the first and last chunks
    store in quarters so the store pipe starts ~2us earlier and the tail
    after the final drain is short.
  - The PE clock ramps 1.2->2.4 GHz only after ~12-19us of sustained
    activity (HAM), and the cold-PE column rate is BELOW the store line
    rate, so dummy warm-up matmuls run from the earliest possible point
    (before the TileContext entry barrier) through the input-load window.
  - A BIR post-pass removes the per-matmul Ldweights reload when
    consecutive matmuls share the same stationary operand (walrus ships
    with --enable-ldw-opt=false), taking the warm matmul cadence from
    ~350ns to ~258ns per 512 columns.

Each one-hot column has exactly one nonzero, so the matmul reproduces
r_weight * k exactly up to bf16 rounding of w, k, and the product.
"""

import numpy as np
import ml_dtypes

BF16 = ml_dtypes.bfloat16

# Problem shape (hardcoded per contest rules).
N, P2, TOPK, W2, CK = 16, 49, 8, 64, 128
NCORES = 8
NB = N // NCORES          # batch elements per core = 2
KROWS = NB * P2           # data rows (contraction dim) per core = 98
ROWS = 128                # padded to 128 partitions: a 98-partition DMA
                          # leaves 2 of every 16 SDMA engines idle and
                          # measured only ~235 GB/s on the load
PT = NB * P2 * TOPK       # output windows per core = 784
WC = W2 * CK              # window elements = 8192
PT_CHUNK = 112            # 7 pt chunks of 112 (<=128 partitions)
MM_CHUNK = 512            # matmul moving free size (this walrus build
                          # rejects 1024-wide Matmult: "ISA check failed")
DR_CHUNK = 1024           # drain free size (one [112,1024] PSUM tile, 2
                          # banks).  2048-wide drains with 2 PSUM slots
                          # measured ~8us worse even with a warm PE: the
                          # coarse PSUM recycling stalls the PE.
ST_CHUNK = 4096           # store free size (two drains per half-store)
LD_SPLIT = 4096           # k columns in the first load

_PROGRAM_CACHE = {}


def _drain_engine_schedule():
    """Assign each of the 56 drains to ScalarE (ACT) or VectorE (DVE).

    Cost per [112,1024] drain: ACT (1024+172)/1.2GHz = 997ns,
    DVE (1024+120)/0.96GHz = 1192ns.  Balanced split: ACT share
    f = 1192/(997+1192) = 0.545 -> 30 ACT / 26 DVE, interleaved.
    """
    n = (PT // PT_CHUNK) * (WC // DR_CHUNK)
    # ACT (1024+172)/1.2GHz = 997ns, DVE (1024+120)/0.96GHz = 1192ns.
    n_act = round(n * 1192.0 / (997.0 + 1192.0))
    sched = []
    acc = 0
    for j in range(n):
        nxt = (j + 1) * n_act // n
        sched.append("act" if nxt > acc else "dve")
        acc = nxt
    return sched


def _build_program(patch=True):
    """Build the (data-independent) per-core Bass program.

    patch=True applies _split_multi_waits (required for the HW compile;
    the JSON round-trip breaks CoreSim, so use patch=False for sim)."""
    import concourse.bass as bass
    import concourse.mybir as mybir
    import concourse.tile as tile

    nc = bass.Bass()
    # PE warm-up part 1, BEFORE the TileContext: the PE ramps from 1.2 to
    # 2.4 GHz only after ~12-19us of sustained activity (HAM), so every
    # us of earlier PE activity moves the ramp earlier.  Raw (non-Tile)
    # dummy matmuls execute right after the PE's NEFF prologue, ~2.5us
    # before the TileContext entry barrier opens.  Inputs are
    # uninitialized garbage; the PSUM target is freed again before the
    # tile pools allocate (PE program order makes the overlap safe).
    with nc.sbuf_tensor([128, 624], mybir.dt.bfloat16) as pre_sb, \
         nc.psum_tensor([112, 512], mybir.dt.float32) as pre_ps:
        for _ in range(5):
            nc.tensor.matmul(pre_ps[:], lhsT=pre_sb[:, :112],
                             rhs=pre_sb[:, 112:624], start=True, stop=True)
    # onehot and k_core are packed into one input ([128, 784+8192]) so a
    # load covers whole partition lines.
    koh_d = nc.dram_tensor("koh", [ROWS, PT + WC], mybir.dt.bfloat16,
                           kind="ExternalInput")
    out_d = nc.dram_tensor("out_core", [PT, WC], mybir.dt.bfloat16,
                           kind="ExternalOutput")

    bf16 = mybir.dt.bfloat16
    f32 = mybir.dt.float32
    n_cp = PT // PT_CHUNK
    n_st = WC // ST_CHUNK
    dr_per_st = ST_CHUNK // DR_CHUNK
    mm_per_dr = DR_CHUNK // MM_CHUNK
    sched = _drain_engine_schedule()

    with tile.TileContext(nc) as tc:
        with (
            tc.tile_pool(name="const", bufs=1) as cpool,
            tc.tile_pool(name="stage", bufs=5) as spool,
            tc.tile_pool(name="psum", bufs=4, space="PSUM") as ppool,
        ):
            koh_sb = cpool.tile([ROWS, PT + WC], bf16)
            # PE warm-up part 2: keep the PE busy through the input-load
            # window so the HAM activity clock keeps running.  Tile's
            # dependency tracker rejects reads of never-written tiles, so
            # the inputs are memset first.
            wu_lhsT = cpool.tile([ROWS, PT_CHUNK], bf16)
            wu_rhs = cpool.tile([ROWS, MM_CHUNK], bf16)
            wu_out = cpool.tile([1, 2], f32)
            wu_ps = ppool.tile([PT_CHUNK, DR_CHUNK], f32, space="PSUM",
                               tag="ps")
            # DVE memsets (no ACT table-load dependency, unlike memzero)
            # so the dummy matmuls start as soon as the PE is up; the ACT
            # copy pre-triggers the 1.28us ACT_TABLE_LOAD off the critical
            # path of the first real drain.
            nc.vector.memset(wu_lhsT[:], 0.0)
            nc.vector.memset(wu_rhs[:], 0.0)
            nc.scalar.copy(out=wu_out[:1, 1:2], in_=wu_rhs[:1, :1])
            for _ in range(7):
                nc.tensor.matmul(wu_ps[:, :MM_CHUNK], lhsT=wu_lhsT[:],
                                 rhs=wu_rhs[:], start=True, stop=True)
            # one tiny read so the pool slot is freed for the main loop
            nc.vector.tensor_copy(out=wu_out[:1, :1], in_=wu_ps[:1, :1])
            # Split loads so matmuls can start after the first ~1 MB.
            # Both issue back-to-back on one HWDGE ring and execute in
            # FIFO order, so load-a streams at full rate and load-b
            # follows with no completion-handoff gap.  (An explicit WAW
            # serialization measured ~3us of idle DMA per handoff; a
            # 3-way serialized split was worse still.)
            # ...and the loads go on the ACT HWDGE ring (stores issue
            # from Sync): per-ring FIFO would otherwise hold the first
            # stores' packets behind load-b's tail even when their data
            # is ready ~1.5us before load-b completes.
            cuts = [0, PT + LD_SPLIT, PT + WC]
            for lo, hi in zip(cuts, cuts[1:]):
                nc.scalar.dma_start(out=koh_sb[:, lo:hi],
                                    in_=koh_d[:, lo:hi])

            di = 0
            ecost = {"act": 0.0, "dve": 0.0}
            for cp in range(n_cp):
                stage = spool.tile([PT_CHUNK, WC], bf16)
                lhsT = koh_sb[:, cp * PT_CHUNK:(cp + 1) * PT_CHUNK]
                # Finer stores on the first chunk start the store stream
                # earlier (fewer drains gate the first store); on the last
                # chunk they shorten the tail.
                st_chunk = ST_CHUNK if cp < n_cp - 1 else ST_CHUNK // 2
                for st in range(WC // st_chunk):
                    # Drains alternate engines per-PSUM-tile (tying all of
                    # one store's drains to a single engine measured ~6us
                    # worse: with 4 PSUM slots it serializes the two drain
                    # engines instead of overlapping them).
                    for dr in range(st_chunk // DR_CHUNK):
                        ps = ppool.tile([PT_CHUNK, DR_CHUNK], f32,
                                        space="PSUM", tag="ps")
                        base = st * st_chunk + dr * DR_CHUNK
                        for m in range(mm_per_dr):
                            rhs = koh_sb[:, PT + base + m * MM_CHUNK:
                                         PT + base + (m + 1) * MM_CHUNK]
                            nc.tensor.matmul(
                                ps[:, m * MM_CHUNK:(m + 1) * MM_CHUNK],
                                lhsT=lhsT, rhs=rhs, start=True, stop=True)
                        sl = slice(base, base + DR_CHUNK)
                        if sched[di] == "act":
                            nc.scalar.copy(out=stage[:, sl], in_=ps[:])
                        else:
                            nc.vector.tensor_copy(out=stage[:, sl], in_=ps[:])
                        di += 1
                    rows = slice(cp * PT_CHUNK, (cp + 1) * PT_CHUNK)
                    cols = slice(st * st_chunk, (st + 1) * st_chunk)
                    nc.sync.dma_start(out=out_d[rows, cols],
                                      in_=stage[:, cols])
    if patch:
        _dedup_ldweights(nc)
        _split_multi_waits(nc)
    return nc


def _dedup_ldweights(nc):
    """Drop the stationary operand from consecutive PE Matmults that use
    identical weights.  Walrus lowers every 2-input Matmult into an
    LDWEIGHTS+MATMUL pair (and bass passes --enable-ldw-opt=false), which
    costs ~140ns of PE time per matmul; the 16 matmuls of one pt-chunk all
    share the same lhsT, so 15 of the 16 weight loads are redundant.  A
    1-input Matmult reuses whatever the PE array already holds (the
    documented bf16 InstLdweights + non-self-loading InstMatmult pairing)."""
    import json
    from concourse import mybir

    j = json.loads(mybir.module_to_json_string(nc.m))
    ndrop = [0]
    for f in j["functions"]:
        for b in f["blocks"]:
            out = []
            prev_w = None
            for ins in b["instructions"]:
                if ins.get("engine") == "PE":
                    op = ins.get("opcode")
                    if op == "Ldweights":
                        w = json.dumps(ins["ins"], sort_keys=True)
                        if w == prev_w:
                            # redundant reload: drop it, but keep its
                            # sync as a standalone semaphore op
                            sync = ins.get("sync_info") or {}
                            if sync.get("on_wait") or sync.get("on_update"):
                                out.append({
                                    "debug": ins.get("debug", 0),
                                    "engine": "PE",
                                    "ins": [],
                                    "name": f"ldwdrop-{ins['name']}",
                                    "opcode": "EventSemaphore",
                                    "outs": [],
                                    "sync_info": sync,
                                })
                            ndrop[0] += 1
                            continue
                        prev_w = w
                    elif op not in ("Matmult", "EventSemaphore", "Nop"):
                        prev_w = None
                out.append(ins)
            b["instructions"] = out
    nc.m = mybir.parse(j)
    return ndrop[0]


def _split_multi_waits(nc):
    """This walrus build rejects >1 fused sync-wait per instruction
    ("Too many sync wait commands"). Tile's wait assigner happily fuses
    several. Rewrite the BIR: for any instruction with N>1 waits, emit
    N-1 standalone single-wait EventSemaphore instructions (same engine,
    immediately before it) and keep only the last wait fused."""
    import json
    from concourse import mybir

    j = json.loads(mybir.module_to_json_string(nc.m))
    uid = [0]
    for f in j["functions"]:
        for b in f["blocks"]:
            out = []
            for ins in b["instructions"]:
                sync = ins.get("sync_info") or {}
                waits = sync.get("on_wait") or []
                if len(waits) > 1:
                    for w in waits[:-1]:
                        uid[0] += 1
                        out.append({
                            "debug": ins.get("debug", 0),
                            "engine": ins["engine"],
                            "ins": [],
                            "name": f"wsplit-{uid[0]}-{ins['name']}",
                            "opcode": "EventSemaphore",
                            "outs": [],
                            "sync_info": {"on_update": [], "on_wait": [w]},
                        })
                    sync["on_wait"] = [waits[-1]]
                out.append(ins)
            b["instructions"] = out
    nc.m = mybir.parse(j)


def get_program():
    if "nc" not in _PROGRAM_CACHE:
        _PROGRAM_CACHE["nc"] = _build_program()
    return _PROGRAM_CACHE["nc"]


def build_in_maps(r_idx, r_weight, k):
    """Host-side sharding + preprocessing: per-core inputs for the program."""
    r_idx = np.asarray(r_idx).astype(np.int64)
    r_weight = np.asarray(r_weight).astype(BF16)
    k = np.asarray(k).astype(BF16)

    pt = np.arange(PT)
    n_l = pt // (P2 * TOPK)
    p = (pt // TOPK) % P2
    t = pt % TOPK

    in_maps = []
    for c in range(NCORES):
        n0 = c * NB
        idx = r_idx[n0:n0 + NB]
        wgt = r_weight[n0:n0 + NB]
        koh = np.zeros((ROWS, PT + WC), BF16)
        rows = n_l * P2 + idx[n_l, p, t]
        koh[rows, pt] = wgt[n_l, p, t]
        koh[:KROWS, PT:] = k[n0:n0 + NB].reshape(KROWS, WC)
        in_maps.append({"koh": koh})
    return in_maps


def run_program(in_maps, trace=False, **kwargs):
    from concourse.bass_utils import run_bass_kernel_spmd
    return run_bass_kernel_spmd(get_program(), in_maps,
                                list(range(NCORES)), trace=trace, **kwargs)


def assemble_output(results):
    out = np.empty((N, P2, TOPK, W2, CK), np.float32)
    for c in range(NCORES):
        out[c * NB:(c + 1) * NB] = np.asarray(
            results[c]["out_core"]).astype(np.float32).reshape(
            NB, P2, TOPK, W2, CK)
    return out


def kernel(r_idx, r_weight, k):
    in_maps = build_in_maps(r_idx, r_weight, k)
    res = run_program(in_maps)
    return assemble_output(res.results)
